# revision 31
# baseline (speedup 1.0000x reference)
"""Trainium2 Bass kernel: per-sample hypernetwork depthwise 3x3 conv.

Reference computation (per batch b):
    W_dw[b] = (z[b] @ W_lin.T).reshape(OUT_C, 1, 3, 3)
    y[b]    = depthwise_conv2d(x[b], W_dw[b], padding=1)

Sharding: data-parallel over batch across 8 NeuronCores (2 batches/core),
W_lin replicated. Each core computes its own W_dw on-device.

Per-core design (v3):
  - x is pre-cast to fp16 and zero-padded to width 130 on the host (the
    device-side compute was already fp16 via casting DMAs, so precision is
    unchanged; the pad removes every width-edge wrap correction and the
    fp16 HBM image halves input DMA traffic)
  - y is written fp16 (halves output traffic); host upcasts to f32
  - channels (256) -> 2 groups of 128 on SBUF partitions
  - image split into 32-row bands; each band loaded by a GPSIMD (SWDGE)
    DMA into a 34-row x 130-col fp16 SBUF tile (1 halo row above/below,
    halo rows chained from the previous band's tile via small SBUF->SBUF
    copies; image-boundary rows memset to 0)
  - 9 conv taps per 16-row PSUM group, all edge-safe via the padded tile:
      * ACT: tap t3 (dx=-1) product written directly to PSUM (the
        accumulation-group initializer)
      * DVE: the 3 center-column taps (dx=0) as tensor_scalar products
        + tensor_tensor adds into an fp16 accumulator acc
      * PE: the remaining 5 taps as [4,128] diag matmuls (start=False
        onto ACT's init), exactly one 512-wide PSUM bank per matmul
      * fold of acc into PSUM split 3:1 -- banks 0-2 via PE identity
        matmuls, bank 3 via a DVE tensor_tensor read-modify-write
  - ACT drains each PSUM group to an fp16 out tile (one group late, so
    ACT's psum-init for group g+1 precedes the drain of group g); SyncE
    issues one output DMA per 32-row band (8KB per-partition lines)
  - W_dw computed on-device by 18 small matmuls from a host-side
    re-layout of W_lin (pure permutation/transpose, no host math)
"""

import os
import sys

for _p in ("/opt/trn_rl_repo", "/root/.axon_site", "/root/.axon_site/_ro/trn_rl_repo",
           "/root/.axon_site/_ro/pypackages"):
    if os.path.isdir(_p) and _p not in sys.path:
        sys.path.append(_p)

import numpy as np

import concourse.bass as bass
import concourse.tile as tile
from concourse import bacc, mybir
from concourse import bass_utils
from concourse.alu_op_type import AluOpType

F32 = mybir.dt.float32
F16 = mybir.dt.float16

# problem constants (hardcoded per contract)
B, OUT_C, H, W = 16, 256, 128, 128
K, Z_DIM = 3, 64
N_CORES = 8
B_PER = B // N_CORES          # 2 batches per core
G = OUT_C // 128              # 2 channel groups of 128

WP = W + 2                    # padded width (zero col at 0 and 129)
ROWS_BAND = 32
ROWS_GROUP = 16
N_BANDS = H // ROWS_BAND      # 4
GRP_PER_BAND = ROWS_BAND // ROWS_GROUP  # 2
TILE_ROWS = ROWS_BAND + 2     # 34 (halo above + below)
FLAT = TILE_ROWS * WP         # 4420
GW = ROWS_GROUP * W           # 2048, psum/out elems per group

# taps: t = dy*3 + dx with offsets (dy-1, dx-1)
ACT_TAP = 3                          # (0, -1)
DVE_TAPS = (1, 4, 7)                 # center column (dx offset 0)
PE_TAPS = (0, 2, 5, 6, 8)            # the 4 corners + (0, +1)

FOLD_PE_BANKS = 3                    # acc fold: banks 0-2 on PE, bank 3 DVE


def build_nc():
    nc = bacc.Bacc("TRN2", target_bir_lowering=False, debug=False)

    x_d = nc.dram_tensor("x", [B_PER, OUT_C, H, WP], F16, kind="ExternalInput")
    zt_d = nc.dram_tensor("zT", [Z_DIM, B_PER], F32, kind="ExternalInput")
    wlt_d = nc.dram_tensor("wlt", [Z_DIM, OUT_C * K * K], F32, kind="ExternalInput")
    ident_d = nc.dram_tensor("ident", [128, 128], F32, kind="ExternalInput")
    y_d = nc.dram_tensor("y", [B_PER, OUT_C, H, W], F16, kind="ExternalOutput")

    n_chunks = OUT_C * K * K // 128          # 18
    wd_cols = K * K * G * B_PER              # 36, col = (g*9 + t)*b_per + b

    with tile.TileContext(nc) as tc:
        with tc.tile_pool(name="wconst", bufs=1) as wpool:
            # weight loads on the gpsimd queue, emitted before the big band
            # transfers swamp the DMA engines (fp16 casts; precision ~5e-4)
            wlt16 = wpool.tile([Z_DIM, OUT_C * K * K], F16)
            half = OUT_C * K * K // 2
            # first half (g=0 tap weights) gates the start; second half can
            # trail behind the first band loads
            nc.gpsimd.dma_start(wlt16[:, 0:half], wlt_d.ap()[:, 0:half])
            ztf = wpool.tile([Z_DIM, B_PER], F32)
            nc.sync.dma_start(ztf[:], zt_d.ap()[:, :])
            zt16 = wpool.tile([Z_DIM, B_PER], F16)
            nc.vector.tensor_scalar(out=zt16[:], in0=ztf[:], scalar1=1.0,
                                    scalar2=None, op0=AluOpType.mult)
            ident = wpool.tile([128, 128], F32)
            nc.sync.dma_start(ident[:], ident_d.ap()[:, :])
            nc.gpsimd.dma_start(wlt16[:, half:], wlt_d.ap()[:, half:])

            wd = wpool.tile([128, wd_cols], F32)

            def wcol(b, g, t):
                return (g * K * K + t) * B_PER + b

            # fp16 diag weights for the PE taps
            identf16 = wpool.tile([128, 128], F16)
            nc.vector.tensor_scalar(out=identf16[:], in0=ident[:], scalar1=1.0,
                                    scalar2=None, op0=AluOpType.mult)
            diags = {}

            def make_diags(b, g):
                for t in PE_TAPS:
                    dt_ = wpool.tile([128, 128], F16, tag=f"d_{b}_{g}_{t}", name="dt_")
                    nc.vector.tensor_scalar(
                        out=dt_[:], in0=ident[:], scalar1=wd[:, wcol(b, g, t):wcol(b, g, t) + 1],
                        scalar2=None, op0=AluOpType.mult)
                    diags[(b, g, t)] = dt_

            with tc.tile_pool(name="xband", bufs=7) as xpool, \
                 tc.tile_pool(name="oband", bufs=4) as opool, \
                 tc.tile_pool(name="accp", bufs=3) as accpool, \
                 tc.tile_pool(name="scrp", bufs=2) as scrpool, \
                 tc.tile_pool(name="psum", bufs=2, space="PSUM") as pspool:

                # PSUM has_written warmup: only TensorE matmuls set the
                # per-element has_written bit; a start=False matmul
                # OVERWRITES where the bit is clear. The main loop relies on
                # ACT writing the psum init with PE accumulating on top
                # (start=False), which only works once every element of both
                # psum bufs has been matmul-written. Do that once up front.
                # stationary/moving dummies are memset (not ident-derived) so
                # the warmup isn't gated on any DMA and leads the PE queue
                dummy = wpool.tile([128, 512], F16)
                nc.vector.memset(dummy[:], 0.0)
                dummy_w = wpool.tile([128, 128], F16)
                nc.vector.memset(dummy_w[:], 0.0)
                for _ in range(2):
                    wt_ = pspool.tile([128, GW], F32, name="ps")
                    for k in range(4):
                        nc.tensor.matmul(wt_[:, 512 * k:512 * (k + 1)],
                                         dummy_w[:], dummy[:],
                                         start=True, stop=True)

                # W_dw chunk matmuls into a pspool tile (after the warmup on
                # the PE queue -- these wait on the wlt/zT DMAs); g=0 chunks
                # drained first so the first (b, g) isn't gated on the rest
                wps_t = pspool.tile([128, GW], F32, name="ps")
                for j in range(n_chunks // 2):
                    nc.tensor.matmul(wps_t[:, B_PER * j:B_PER * (j + 1)],
                                     wlt16[:, 128 * j:128 * (j + 1)], zt16[:],
                                     start=True, stop=True, skip_group_check=True)
                nc.vector.tensor_copy(wd[:, 0:B_PER * (n_chunks // 2)],
                                      wps_t[:, 0:B_PER * (n_chunks // 2)])
                for j in range(n_chunks // 2, n_chunks):
                    nc.tensor.matmul(wps_t[:, B_PER * j:B_PER * (j + 1)],
                                     wlt16[:, 128 * j:128 * (j + 1)], zt16[:],
                                     start=True, stop=True, skip_group_check=True)
                nc.vector.tensor_copy(wd[:, B_PER * (n_chunks // 2):wd_cols],
                                      wps_t[:, B_PER * (n_chunks // 2):wd_cols])
                # the chunk matmuls' start=True groups invalidated this
                # bank's has_written coverage -- re-warm it for the main loop
                nc.tensor.matmul(wps_t[:, 0:512], dummy_w[:], dummy[:],
                                 start=True, stop=True, skip_group_check=True)

                # (b, g, band) walk order, for one-band-ahead prefetch
                walk = [(b, g, band)
                        for b in range(B_PER) for g in range(G)
                        for band in range(N_BANDS)]
                xb_tiles = {}

                def emit_load(idx):
                    """Issue halo + main DMA for walk[idx]."""
                    b, g, band = walk[idx]
                    r0 = band * ROWS_BAND
                    xb = xpool.tile([128, FLAT], F16)
                    if band == 0:
                        nc.gpsimd.memset(xb[:, 0:WP], 0.0)
                        lo, dst0 = 0, WP
                    else:
                        prev_xb = xb_tiles[idx - 1]
                        nc.sync.dma_start(xb[:, 0:2 * WP],
                                          prev_xb[:, ROWS_BAND * WP:(ROWS_BAND + 2) * WP])
                        lo, dst0 = r0 + 1, 2 * WP
                    hi = min(r0 + ROWS_BAND + 1, H)
                    if band == N_BANDS - 1:
                        nc.gpsimd.memset(xb[:, (TILE_ROWS - 1) * WP:], 0.0)
                    # main band load: fp16 HBM -> fp16 SBUF
                    if idx == 0:
                        nc.gpsimd.dma_start(
                            xb[:, WP:19 * WP],
                            x_d.ap()[b, 0:128, 0:18, :])
                        nc.gpsimd.dma_start(
                            xb[:, 19 * WP:34 * WP],
                            x_d.ap()[b, 0:128, 18:33, :])
                    else:
                        nc.gpsimd.dma_start(
                            xb[:, dst0:dst0 + (hi - lo) * WP],
                            x_d.ap()[b, 128 * g:128 * (g + 1), lo:hi, :])
                    xb_tiles[idx] = xb[:]

                emit_load(0)
                if len(walk) > 1:
                    emit_load(1)
                # pending tail per group, emitted one group late:
                #   PE: fold acc banks 0..2 (identity matmuls, stop on last)
                #   DVE: fold acc bank 3 (psum read-modify-write add)
                #   ACT: drain psum -> fp16 out tile half
                #   sync: band out DMA after the band's 2nd group
                pending = []

                def flush_pending():
                    while pending:
                        ps_, acc_, ot_ap, dma = pending.pop(0)
                        for k in range(FOLD_PE_BANKS):
                            nc.tensor.matmul(
                                ps_[:, 512 * k:512 * (k + 1)], identf16[:],
                                acc_[:, 512 * k:512 * (k + 1)],
                                start=False, stop=(k == FOLD_PE_BANKS - 1),
                                skip_group_check=True)
                        nc.vector.tensor_tensor(
                            out=ps_[:, 512 * FOLD_PE_BANKS:],
                            in0=ps_[:, 512 * FOLD_PE_BANKS:],
                            in1=acc_[:, 512 * FOLD_PE_BANKS:],
                            op=AluOpType.add)
                        nc.scalar.copy(ot_ap, ps_[:])
                        if dma is not None:
                            nc.sync.dma_start(dma[0], dma[1])

                for idx, (b, g, band) in enumerate(walk):
                    if band == 0:
                        make_diags(b, g)
                    r0 = band * ROWS_BAND
                    xb = xb_tiles[idx]
                    xbv = xb.rearrange("p (t c) -> p t c", c=WP)
                    # prefetch two bands ahead so transfers stay ahead of
                    # compute even through the pipeline ramp
                    if idx + 2 < len(walk):
                        emit_load(idx + 2)
                    xb_tiles.pop(idx - 1, None)

                    ot = opool.tile([128, ROWS_BAND * W], F16)

                    def tap_ap(t, row0, nrows):
                        dy, dx = t // 3 - 1, t % 3 - 1
                        return xbv[:, row0 + dy:row0 + dy + nrows,
                                   1 + dx:129 + dx]

                    for grp in range(GRP_PER_BAND):
                        g0 = grp * ROWS_GROUP + 1   # tile row of first out row
                        ps = pspool.tile([128, GW], F32, name="ps")
                        psv = ps[:].rearrange("p (t c) -> p t c", c=W)

                        # ---- ACT: tap t3 product -> psum (init) ----
                        nc.scalar.mul(psv[:, :, :], tap_ap(ACT_TAP, g0, ROWS_GROUP),
                                      wd[:, wcol(b, g, ACT_TAP):wcol(b, g, ACT_TAP) + 1])

                        # ---- previous group's fold + drain + DMA ----
                        flush_pending()

                        # ---- DVE: 3 center taps -> fp16 acc ----
                        acc = accpool.tile([128, GW], F16)
                        scr = scrpool.tile([128, GW], F16)
                        accv = acc[:].rearrange("p (t c) -> p t c", c=W)
                        scrv = scr[:].rearrange("p (t c) -> p t c", c=W)
                        t0, t1_, t2_ = DVE_TAPS
                        nc.vector.tensor_scalar(
                            out=accv[:, :, :], in0=tap_ap(t0, g0, ROWS_GROUP),
                            scalar1=wd[:, wcol(b, g, t0):wcol(b, g, t0) + 1],
                            scalar2=None, op0=AluOpType.mult)
                        nc.vector.tensor_scalar(
                            out=scrv[:, :, :], in0=tap_ap(t1_, g0, ROWS_GROUP),
                            scalar1=wd[:, wcol(b, g, t1_):wcol(b, g, t1_) + 1],
                            scalar2=None, op0=AluOpType.mult)
                        nc.vector.tensor_tensor(out=acc[:], in0=acc[:], in1=scr[:],
                                                op=AluOpType.add)
                        nc.vector.tensor_scalar(
                            out=scrv[:, :, :], in0=tap_ap(t2_, g0, ROWS_GROUP),
                            scalar1=wd[:, wcol(b, g, t2_):wcol(b, g, t2_) + 1],
                            scalar2=None, op0=AluOpType.mult)
                        nc.vector.tensor_tensor(out=acc[:], in0=acc[:], in1=scr[:],
                                                op=AluOpType.add)

                        # ---- PE: 5 taps, [4,128] chunks, one bank each ----
                        for t in PE_TAPS:
                            for k in range(4):
                                nc.tensor.matmul(
                                    psv[:, 4 * k:4 * k + 4, :],
                                    diags[(b, g, t)][:],
                                    tap_ap(t, g0 + 4 * k, 4),
                                    start=False, stop=False,
                                    skip_group_check=True)

                        # ---- queue the group tail (see flush_pending) ----
                        # last band: per-group DMAs so the tail pipeline
                        # overlaps the final drains
                        dma = None
                        if idx == len(walk) - 1:
                            r_out = r0 + grp * ROWS_GROUP
                            dma = (y_d.ap()[b, 128 * g:128 * (g + 1),
                                            r_out:r_out + ROWS_GROUP, :],
                                   ot[:, grp * GW:(grp + 1) * GW])
                        elif grp == GRP_PER_BAND - 1:
                            dma = (y_d.ap()[b, 128 * g:128 * (g + 1),
                                            r0:r0 + ROWS_BAND, :], ot[:])
                        pending.append(
                            (ps, acc[:],
                             ot[:, grp * GW:(grp + 1) * GW],
                             dma))
                flush_pending()

    nc.compile()
    return nc


def make_in_maps(x, z, W_lin, b_per=B_PER):
    """Host-side shard + layout transforms (fp16 cast + zero-pad of x)."""
    wl = np.asarray(W_lin, dtype=np.float32)
    wlperm = (wl.reshape(G, 128, K * K, Z_DIM)
                .transpose(0, 2, 1, 3)
                .reshape(OUT_C * K * K, Z_DIM))
    wlt = np.ascontiguousarray(wlperm.T)                  # [64, 2304]
    ident = np.eye(128, dtype=np.float32)
    xp = np.zeros((B, OUT_C, H, WP), dtype=np.float16)
    xp[..., 1:1 + W] = np.asarray(x)
    z = np.asarray(z, dtype=np.float32)
    in_maps = []
    for c in range(N_CORES):
        sl = slice(c * b_per, (c + 1) * b_per)
        in_maps.append({
            "x": np.ascontiguousarray(xp[sl]),
            "zT": np.ascontiguousarray(z[sl].T),          # [64, b_per]
            "wlt": wlt,
            "ident": ident,
        })
    return in_maps


_NC_CACHE = {}


def kernel(x, z, W_lin):
    key = "main"
    if key not in _NC_CACHE:
        _NC_CACHE[key] = build_nc()
    nc = _NC_CACHE[key]
    in_maps = make_in_maps(x, z, W_lin)
    res = bass_utils.run_bass_kernel_spmd(nc, in_maps, core_ids=list(range(N_CORES)))
    out = np.concatenate([res.results[c]["y"] for c in range(N_CORES)], axis=0)
    return out.astype(np.float32, copy=False)


# revision 33
# speedup vs baseline: 1.0039x; 1.0039x over previous
"""Trainium2 Bass kernel: per-sample hypernetwork depthwise 3x3 conv.

Reference computation (per batch b):
    W_dw[b] = (z[b] @ W_lin.T).reshape(OUT_C, 1, 3, 3)
    y[b]    = depthwise_conv2d(x[b], W_dw[b], padding=1)

Sharding: data-parallel over batch across 8 NeuronCores (2 batches/core),
W_lin replicated. Each core computes its own W_dw on-device.

Per-core design (v3):
  - x is pre-cast to fp16 and zero-padded to width 130 on the host (the
    device-side compute was already fp16 via casting DMAs, so precision is
    unchanged; the pad removes every width-edge wrap correction and the
    fp16 HBM image halves input DMA traffic)
  - y is written fp16 (halves output traffic); host upcasts to f32
  - channels (256) -> 2 groups of 128 on SBUF partitions
  - image split into 32-row bands; each band loaded by a GPSIMD (SWDGE)
    DMA into a 34-row x 130-col fp16 SBUF tile (1 halo row above/below,
    halo rows chained from the previous band's tile via small SBUF->SBUF
    copies; image-boundary rows memset to 0)
  - 9 conv taps per 16-row PSUM group, all edge-safe via the padded tile:
      * ACT: tap t3 (dx=-1) product written directly to PSUM (the
        accumulation-group initializer)
      * DVE: the 3 center-column taps (dx=0) as tensor_scalar products
        + tensor_tensor adds into an fp16 accumulator acc
      * PE: the remaining 5 taps as [4,128] diag matmuls (start=False
        onto ACT's init), exactly one 512-wide PSUM bank per matmul
      * fold of acc into PSUM split 3:1 -- banks 0-2 via PE identity
        matmuls, bank 3 via a DVE tensor_tensor read-modify-write
  - ACT drains each PSUM group to an fp16 out tile (one group late, so
    ACT's psum-init for group g+1 precedes the drain of group g); SyncE
    issues one output DMA per 32-row band (8KB per-partition lines)
  - W_dw computed on-device by 18 small matmuls from a host-side
    re-layout of W_lin (pure permutation/transpose, no host math)
"""

import os
import sys

for _p in ("/opt/trn_rl_repo", "/root/.axon_site", "/root/.axon_site/_ro/trn_rl_repo",
           "/root/.axon_site/_ro/pypackages"):
    if os.path.isdir(_p) and _p not in sys.path:
        sys.path.append(_p)

import numpy as np

import concourse.bass as bass
import concourse.tile as tile
from concourse import bacc, mybir
from concourse import bass_utils
from concourse.alu_op_type import AluOpType

F32 = mybir.dt.float32
F16 = mybir.dt.float16

# problem constants (hardcoded per contract)
B, OUT_C, H, W = 16, 256, 128, 128
K, Z_DIM = 3, 64
N_CORES = 8
B_PER = B // N_CORES          # 2 batches per core
G = OUT_C // 128              # 2 channel groups of 128

WP = W + 2                    # padded width (zero col at 0 and 129)
ROWS_BAND = 32
ROWS_GROUP = 16
N_BANDS = H // ROWS_BAND      # 4
GRP_PER_BAND = ROWS_BAND // ROWS_GROUP  # 2
TILE_ROWS = ROWS_BAND + 2     # 34 (halo above + below)
FLAT = TILE_ROWS * WP         # 4420
GW = ROWS_GROUP * W           # 2048, psum/out elems per group

# taps: t = dy*3 + dx with offsets (dy-1, dx-1)
ACT_TAP = 3                          # (0, -1)
DVE_TAPS = (1, 4, 7)                 # center column (dx offset 0)
PE_TAPS = (0, 2, 5, 6, 8)            # the 4 corners + (0, +1)

FOLD_PE_BANKS = 3                    # acc fold: banks 0-2 on PE, bank 3 DVE


def build_nc():
    nc = bacc.Bacc("TRN2", target_bir_lowering=False, debug=False)

    x_d = nc.dram_tensor("x", [B_PER, OUT_C, H, WP], F16, kind="ExternalInput")
    zt_d = nc.dram_tensor("zT", [Z_DIM, B_PER], F32, kind="ExternalInput")
    wlt_d = nc.dram_tensor("wlt", [Z_DIM, OUT_C * K * K], F32, kind="ExternalInput")
    ident_d = nc.dram_tensor("ident", [128, 128], F32, kind="ExternalInput")
    y_d = nc.dram_tensor("y", [B_PER, OUT_C, H, W], F16, kind="ExternalOutput")

    n_chunks = OUT_C * K * K // 128          # 18
    wd_cols = K * K * G * B_PER              # 36, col = (g*9 + t)*b_per + b

    with tile.TileContext(nc) as tc:
        with tc.tile_pool(name="wconst", bufs=1) as wpool:
            # weight loads on the gpsimd queue, emitted before the big band
            # transfers swamp the DMA engines (fp16 casts; precision ~5e-4)
            wlt16 = wpool.tile([Z_DIM, OUT_C * K * K], F16)
            half = OUT_C * K * K // 2
            # first half (g=0 tap weights) gates the start; second half can
            # trail behind the first band loads
            nc.gpsimd.dma_start(wlt16[:, 0:half], wlt_d.ap()[:, 0:half])
            ztf = wpool.tile([Z_DIM, B_PER], F32)
            nc.sync.dma_start(ztf[:], zt_d.ap()[:, :])
            zt16 = wpool.tile([Z_DIM, B_PER], F16)
            nc.vector.tensor_scalar(out=zt16[:], in0=ztf[:], scalar1=1.0,
                                    scalar2=None, op0=AluOpType.mult)
            ident = wpool.tile([128, 128], F32)
            nc.sync.dma_start(ident[:], ident_d.ap()[:, :])
            nc.gpsimd.dma_start(wlt16[:, half:], wlt_d.ap()[:, half:])

            wd = wpool.tile([128, wd_cols], F32)
            with tc.tile_pool(name="wpsum", bufs=1, space="PSUM") as wps:
                # chunk matmuls into one PSUM bank; g=0 chunks drained first
                # so the main loop's first (b, g) isn't gated on the rest
                ps = wps.tile([128, wd_cols], F32)
                for j in range(n_chunks // 2):
                    nc.tensor.matmul(ps[:, B_PER * j:B_PER * (j + 1)],
                                     wlt16[:, 128 * j:128 * (j + 1)], zt16[:],
                                     start=True, stop=True, skip_group_check=True)
                nc.vector.tensor_copy(wd[:, 0:B_PER * (n_chunks // 2)],
                                      ps[:, 0:B_PER * (n_chunks // 2)])
                for j in range(n_chunks // 2, n_chunks):
                    nc.tensor.matmul(ps[:, B_PER * j:B_PER * (j + 1)],
                                     wlt16[:, 128 * j:128 * (j + 1)], zt16[:],
                                     start=True, stop=True, skip_group_check=True)
                nc.vector.tensor_copy(wd[:, B_PER * (n_chunks // 2):wd_cols],
                                      ps[:, B_PER * (n_chunks // 2):wd_cols])

            def wcol(b, g, t):
                return (g * K * K + t) * B_PER + b

            # fp16 diag weights for the PE taps
            identf16 = wpool.tile([128, 128], F16)
            nc.vector.tensor_scalar(out=identf16[:], in0=ident[:], scalar1=1.0,
                                    scalar2=None, op0=AluOpType.mult)
            diags = {}

            def make_diags(b, g):
                for t in PE_TAPS:
                    dt_ = wpool.tile([128, 128], F16, tag=f"d_{b}_{g}_{t}", name="dt_")
                    nc.vector.tensor_scalar(
                        out=dt_[:], in0=ident[:], scalar1=wd[:, wcol(b, g, t):wcol(b, g, t) + 1],
                        scalar2=None, op0=AluOpType.mult)
                    diags[(b, g, t)] = dt_

            with tc.tile_pool(name="xband", bufs=7) as xpool, \
                 tc.tile_pool(name="oband", bufs=4) as opool, \
                 tc.tile_pool(name="accp", bufs=3) as accpool, \
                 tc.tile_pool(name="scrp", bufs=2) as scrpool, \
                 tc.tile_pool(name="psum", bufs=2, space="PSUM") as pspool:

                # PSUM has_written warmup: only TensorE matmuls set the
                # per-element has_written bit; a start=False matmul
                # OVERWRITES where the bit is clear. The main loop relies on
                # ACT writing the psum init with PE accumulating on top
                # (start=False), which only works once every element of both
                # psum bufs has been matmul-written. Do that once up front.
                # stationary/moving dummies are memset (not ident-derived) so
                # the warmup isn't gated on any DMA and leads the PE queue
                dummy = wpool.tile([128, 512], F16)
                nc.vector.memset(dummy[:], 0.0)
                dummy_w = wpool.tile([128, 128], F16)
                nc.vector.memset(dummy_w[:], 0.0)
                for _ in range(2):
                    wt_ = pspool.tile([128, GW], F32, name="ps")
                    for k in range(4):
                        nc.tensor.matmul(wt_[:, 512 * k:512 * (k + 1)],
                                         dummy_w[:], dummy[:],
                                         start=True, stop=True)

                # (b, g, band) walk order, for one-band-ahead prefetch
                walk = [(b, g, band)
                        for b in range(B_PER) for g in range(G)
                        for band in range(N_BANDS)]
                xb_tiles = {}

                def emit_load(idx):
                    """Issue halo + main DMA for walk[idx]."""
                    b, g, band = walk[idx]
                    r0 = band * ROWS_BAND
                    xb = xpool.tile([128, FLAT], F16)
                    if band == 0:
                        nc.gpsimd.memset(xb[:, 0:WP], 0.0)
                        lo, dst0 = 0, WP
                    else:
                        prev_xb = xb_tiles[idx - 1]
                        nc.sync.dma_start(xb[:, 0:2 * WP],
                                          prev_xb[:, ROWS_BAND * WP:(ROWS_BAND + 2) * WP])
                        lo, dst0 = r0 + 1, 2 * WP
                    hi = min(r0 + ROWS_BAND + 1, H)
                    if band == N_BANDS - 1:
                        nc.gpsimd.memset(xb[:, (TILE_ROWS - 1) * WP:], 0.0)
                    # main band load: fp16 HBM -> fp16 SBUF
                    if idx == 0:
                        nc.gpsimd.dma_start(
                            xb[:, WP:19 * WP],
                            x_d.ap()[b, 0:128, 0:18, :])
                        nc.gpsimd.dma_start(
                            xb[:, 19 * WP:34 * WP],
                            x_d.ap()[b, 0:128, 18:33, :])
                    else:
                        nc.gpsimd.dma_start(
                            xb[:, dst0:dst0 + (hi - lo) * WP],
                            x_d.ap()[b, 128 * g:128 * (g + 1), lo:hi, :])
                    xb_tiles[idx] = xb[:]

                emit_load(0)
                if len(walk) > 1:
                    emit_load(1)
                # pending tail per group, emitted one group late:
                #   PE: fold acc banks 0..2 (identity matmuls, stop on last)
                #   DVE: fold acc bank 3 (psum read-modify-write add)
                #   ACT: drain psum -> fp16 out tile half
                #   sync: band out DMA after the band's 2nd group
                pending = []

                def flush_pending():
                    while pending:
                        ps_, acc_, ot_ap, dma = pending.pop(0)
                        for k in range(FOLD_PE_BANKS):
                            nc.tensor.matmul(
                                ps_[:, 512 * k:512 * (k + 1)], identf16[:],
                                acc_[:, 512 * k:512 * (k + 1)],
                                start=False, stop=(k == FOLD_PE_BANKS - 1),
                                skip_group_check=True)
                        nc.vector.tensor_tensor(
                            out=ps_[:, 512 * FOLD_PE_BANKS:],
                            in0=ps_[:, 512 * FOLD_PE_BANKS:],
                            in1=acc_[:, 512 * FOLD_PE_BANKS:],
                            op=AluOpType.add)
                        nc.scalar.copy(ot_ap, ps_[:])
                        if dma is not None:
                            nc.sync.dma_start(dma[0], dma[1])

                for idx, (b, g, band) in enumerate(walk):
                    if band == 0:
                        make_diags(b, g)
                    r0 = band * ROWS_BAND
                    xb = xb_tiles[idx]
                    xbv = xb.rearrange("p (t c) -> p t c", c=WP)
                    # prefetch two bands ahead so transfers stay ahead of
                    # compute even through the pipeline ramp
                    if idx + 2 < len(walk):
                        emit_load(idx + 2)
                    xb_tiles.pop(idx - 1, None)

                    ot = opool.tile([128, ROWS_BAND * W], F16)

                    def tap_ap(t, row0, nrows):
                        dy, dx = t // 3 - 1, t % 3 - 1
                        return xbv[:, row0 + dy:row0 + dy + nrows,
                                   1 + dx:129 + dx]

                    for grp in range(GRP_PER_BAND):
                        g0 = grp * ROWS_GROUP + 1   # tile row of first out row
                        ps = pspool.tile([128, GW], F32, name="ps")
                        psv = ps[:].rearrange("p (t c) -> p t c", c=W)

                        # ---- ACT: tap t3 product -> psum (init) ----
                        nc.scalar.mul(psv[:, :, :], tap_ap(ACT_TAP, g0, ROWS_GROUP),
                                      wd[:, wcol(b, g, ACT_TAP):wcol(b, g, ACT_TAP) + 1])

                        # ---- previous group's fold + drain + DMA ----
                        flush_pending()

                        # ---- DVE: 3 center taps -> fp16 acc ----
                        acc = accpool.tile([128, GW], F16)
                        scr = scrpool.tile([128, GW], F16)
                        accv = acc[:].rearrange("p (t c) -> p t c", c=W)
                        scrv = scr[:].rearrange("p (t c) -> p t c", c=W)
                        t0, t1_, t2_ = DVE_TAPS
                        nc.vector.tensor_scalar(
                            out=accv[:, :, :], in0=tap_ap(t0, g0, ROWS_GROUP),
                            scalar1=wd[:, wcol(b, g, t0):wcol(b, g, t0) + 1],
                            scalar2=None, op0=AluOpType.mult)
                        nc.vector.tensor_scalar(
                            out=scrv[:, :, :], in0=tap_ap(t1_, g0, ROWS_GROUP),
                            scalar1=wd[:, wcol(b, g, t1_):wcol(b, g, t1_) + 1],
                            scalar2=None, op0=AluOpType.mult)
                        nc.vector.tensor_tensor(out=acc[:], in0=acc[:], in1=scr[:],
                                                op=AluOpType.add)
                        nc.vector.tensor_scalar(
                            out=scrv[:, :, :], in0=tap_ap(t2_, g0, ROWS_GROUP),
                            scalar1=wd[:, wcol(b, g, t2_):wcol(b, g, t2_) + 1],
                            scalar2=None, op0=AluOpType.mult)
                        nc.vector.tensor_tensor(out=acc[:], in0=acc[:], in1=scr[:],
                                                op=AluOpType.add)

                        # ---- PE: 5 taps, [4,128] chunks, one bank each ----
                        for t in PE_TAPS:
                            for k in range(4):
                                nc.tensor.matmul(
                                    psv[:, 4 * k:4 * k + 4, :],
                                    diags[(b, g, t)][:],
                                    tap_ap(t, g0 + 4 * k, 4),
                                    start=False, stop=False,
                                    skip_group_check=True)

                        # ---- queue the group tail (see flush_pending) ----
                        # last band: per-group DMAs so the tail pipeline
                        # overlaps the final drains
                        dma = None
                        if idx == len(walk) - 1:
                            r_out = r0 + grp * ROWS_GROUP
                            dma = (y_d.ap()[b, 128 * g:128 * (g + 1),
                                            r_out:r_out + ROWS_GROUP, :],
                                   ot[:, grp * GW:(grp + 1) * GW])
                        elif grp == GRP_PER_BAND - 1:
                            dma = (y_d.ap()[b, 128 * g:128 * (g + 1),
                                            r0:r0 + ROWS_BAND, :], ot[:])
                        pending.append(
                            (ps, acc[:],
                             ot[:, grp * GW:(grp + 1) * GW],
                             dma))
                flush_pending()

    nc.compile()
    return nc


def make_in_maps(x, z, W_lin, b_per=B_PER):
    """Host-side shard + layout transforms (fp16 cast + zero-pad of x)."""
    wl = np.asarray(W_lin, dtype=np.float32)
    wlperm = (wl.reshape(G, 128, K * K, Z_DIM)
                .transpose(0, 2, 1, 3)
                .reshape(OUT_C * K * K, Z_DIM))
    wlt = np.ascontiguousarray(wlperm.T)                  # [64, 2304]
    ident = np.eye(128, dtype=np.float32)
    xp = np.zeros((B, OUT_C, H, WP), dtype=np.float16)
    xp[..., 1:1 + W] = np.asarray(x)
    z = np.asarray(z, dtype=np.float32)
    in_maps = []
    for c in range(N_CORES):
        sl = slice(c * b_per, (c + 1) * b_per)
        in_maps.append({
            "x": np.ascontiguousarray(xp[sl]),
            "zT": np.ascontiguousarray(z[sl].T),          # [64, b_per]
            "wlt": wlt,
            "ident": ident,
        })
    return in_maps


_NC_CACHE = {}


def kernel(x, z, W_lin):
    key = "main"
    if key not in _NC_CACHE:
        _NC_CACHE[key] = build_nc()
    nc = _NC_CACHE[key]
    in_maps = make_in_maps(x, z, W_lin)
    res = bass_utils.run_bass_kernel_spmd(nc, in_maps, core_ids=list(range(N_CORES)))
    out = np.concatenate([res.results[c]["y"] for c in range(N_CORES)], axis=0)
    return out.astype(np.float32, copy=False)
